# revision 14
# baseline (speedup 1.0000x reference)
"""v3: 4 row-groups x 2 column-halves at 256-col granularity.

C = triu(A @ B), 4096^2, bf16.  Cores (r, s): r = c % 4, s = c // 4.
16 half-columns q' (256 cols, depth 2q'+2 k-tiles).  s=0 takes odd q',
s=1 even.  Env col e = 0..7 pairs ranks: s=0 -> q' = 15-2e (depth
32-4e = DEPTH[e]), s=1 -> q' = 14-2e (depth 30-4e, front-padded 2 zero
k-steps so its diagonal taper aligns with the schedule's).
Slots j = 0..7, block b = 4j + r, live iff 4j < DEPTH[e].
"""

import numpy as np
from contextlib import ExitStack

import concourse.mybir as mybir
import concourse.tile as tile
from concourse import bacc, bass_utils

N = 4096
P = 128
NCORES = 8
CW = 256
NENV = 8
DEPTH = [32 - 4 * e for e in range(NENV)]
NSLOT = 8
PAIRS = [(e, j) for e in range(NENV) for j in range(NSLOT)
         if 4 * j < DEPTH[e]]
NT = len(PAIRS)                          # 36 output tiles per core
# A-pack, w-major: position(w, j) = PRE[w] + j for j <= w//4
PRE = np.cumsum([0] + [w // 4 + 1 for w in range(32)]).tolist()
ATOT = PRE[32]                           # 144 tiles
ACH = [(PRE[4 * kc], PRE[4 * kc + 4]) for kc in range(8)]

BUFS_B = 10
BUFS_O = 6
BUFS_PS = 8

# env col emission order; first FWD_COLS run forward-k.  Ascending size
# start (small DMA deficit while PE ramps), deep cols last (PE-rich per
# byte, so the tail is compute-bound, not DMA-bound).
I_ORDER = [6, 5, 4, 3, 2, 7, 1, 0]
FWD_COLS = 0
# stores of the first DEFER_COLS cols are held in SBUF and issued at the
# last col's start, keeping the DMA-deficit phase free of store traffic
DEFER_COLS = 0

MODE = "bf16"
_nc_cache = {}


def build_nc(mode=None, rep=1, variant="full"):
    if mode in (1, 2, 4, 8):
        rep, mode = mode, None
    key = (rep, variant, tuple(I_ORDER), FWD_COLS, DEFER_COLS)
    if key in _nc_cache:
        return _nc_cache[key]
    dt_in = mybir.dt.bfloat16
    dt_out = mybir.dt.bfloat16

    nc = bacc.Bacc("TRN2", target_bir_lowering=False, debug=False,
                   num_devices=NCORES)
    a_dram = nc.dram_tensor("Apack", [P, ATOT * P], dt_in,
                            kind="ExternalInput").ap()
    # B-pack row = e*P + p, col = w*CW + n  (w local to env col)
    b_dram = nc.dram_tensor("B", [NENV * P, 32 * CW], dt_in,
                            kind="ExternalInput").ap()
    c_dram = nc.dram_tensor("Cout", [NT * P, CW], dt_out,
                            kind="ExternalOutput").ap()

    with tile.TileContext(nc) as tc:
        with ExitStack() as ctx:
            apool = ctx.enter_context(tc.tile_pool(name="apool", bufs=1))
            bpool = ctx.enter_context(tc.tile_pool(name="bpool", bufs=BUFS_B))
            opool = ctx.enter_context(tc.tile_pool(name="opool", bufs=BUFS_O))
            pspool = ctx.enter_context(
                tc.tile_pool(name="pspool", bufs=BUFS_PS, space="PSUM"))

            do_bdma = variant in ("full", "nomm")
            do_mm = variant in ("full", "nodma")

            a_sb = apool.tile([P, ATOT, P], dt_in)

            # per-col plan: groups of up to 2 chunks in processing order
            plan = []
            seen_a = set()
            for ii, e in enumerate(I_ORDER):
                ngk = DEPTH[e] // 4
                fwd = ii < FWD_COLS
                kgs = list(range(ngk)) if fwd \
                    else list(range(ngk - 1, -1, -1))
                groups = []
                for x in range(0, len(kgs), 2):
                    grp = kgs[x:x + 2]
                    anew = [kg for kg in grp if kg not in seen_a]
                    seen_a.update(anew)
                    groups.append((grp, sorted(anew)))
                plan.append((e, ngk, fwd, groups))

            def _issue_a_chunk(kc):
                t0, t1 = ACH[kc]
                nc.sync.dma_start(
                    a_sb[:, t0:t1, :],
                    a_dram[:, t0 * P:t1 * P].rearrange(
                        "p (t m) -> p t m", m=P))

            def _bw(e, w0, w1):
                # B window [w0, w1) of env col e as one DMA source
                return b_dram[
                    e * P:(e + 1) * P,
                    w0 * CW:w1 * CW,
                ].rearrange("p (ko n) -> p ko n", ko=w1 - w0)

            deferred = []   # (t0, nlane, ot) stores held back

            def _store_pair(t0, nlane, ot):
                if nlane == 2:
                    nc.gpsimd.dma_start(
                        c_dram[t0 * P:(t0 + 2) * P, :].rearrange(
                            "(g p) n -> p g n", g=2), ot[:])
                else:
                    nc.gpsimd.dma_start(
                        c_dram[t0 * P:(t0 + 1) * P, :], ot[:, 0])

            def _emit_pair(e, jpair, pst, do_mm, lane0=0, defer=False):
                """One copy + one store for slots jpair sharing a psum
                bank.  jpair ascending; lanes t-ascending by layout.
                lane0: psum lane of jpair[0]."""
                nlane = len(jpair)
                t0 = PAIRS.index((e, jpair[0]))
                ot = opool.tile([P, nlane, CW], dt_out, tag=f"ot{nlane}")
                if do_mm:
                    nc.vector.tensor_copy(
                        ot[:], pst[:, lane0:lane0 + nlane])
                else:
                    nc.vector.tensor_copy(
                        ot[:].rearrange("p g (a b) -> p g a b", a=2),
                        a_sb[:, :2 * nlane, :].rearrange(
                            "p (g a) m -> p g a m", g=nlane))
                if defer:
                    deferred.append((t0, nlane, ot))
                else:
                    _store_pair(t0, nlane, ot)

            bts = {}       # (ci, gi) -> loaded B tile
            bt_fixed = [None]

            def _load_group(ci, gi, first=False):
                if (ci, gi) in bts:
                    return bts[(ci, gi)]
                e, ngk, fwd, groups = plan[ci]
                grp, anew = groups[gi]
                # A chunk feeding the group's first-processed chunk goes
                # before the B load; the rest after (off the critical path)
                a_pre = [kc for kc in anew if kc == grp[0]]
                a_post = [kc for kc in anew if kc != grp[0]]
                for kc in a_pre:
                    _issue_a_chunk(kc)
                w0 = 4 * min(grp)
                w1 = 4 * max(grp) + 4
                if not do_bdma:
                    if bt_fixed[0] is None:
                        bt_fixed[0] = bpool.tile([P, 8, CW], dt_in,
                                                 tag="bt8")
                        nc.sync.dma_start(bt_fixed[0][:], _bw(e, 0, 8))
                    bts[(ci, gi)] = bt_fixed[0]
                    for kc in a_post:
                        _issue_a_chunk(kc)
                    return bt_fixed[0]
                bt = bpool.tile([P, w1 - w0, CW], dt_in,
                                tag=f"bt{w1 - w0}")
                isdiag = max(grp) == ngk - 1
                if first:
                    # first two k-tiles individually (first matmul waits on
                    # 64KB), rest as one load
                    for u in range(2):
                        nc.sync.dma_start(
                            bt[:, u, :],
                            b_dram[e * P:(e + 1) * P,
                                   (w0 + u) * CW:(w0 + u + 1) * CW])
                    last = w1 - 1 - w0
                    if isdiag:
                        nc.sync.dma_start(bt[:, 2:last, :],
                                          _bw(e, w0 + 2, w1 - 1))
                        nc.sync.dma_start(
                            bt[:, last, 128:],
                            b_dram[e * P:(e + 1) * P,
                                   (w1 - 1) * CW + 128:w1 * CW])
                    else:
                        nc.sync.dma_start(bt[:, 2:, :], _bw(e, w0 + 2, w1))
                elif isdiag:
                    # top k-tile's low 128 cols are below-diag zeros
                    nc.sync.dma_start(bt[:, :w1 - 1 - w0, :],
                                      _bw(e, w0, w1 - 1))
                    nc.sync.dma_start(
                        bt[:, w1 - 1 - w0, 128:],
                        b_dram[e * P:(e + 1) * P,
                               (w1 - 1) * CW + 128:w1 * CW])
                else:
                    nc.sync.dma_start(bt[:], _bw(e, w0, w1))
                bts[(ci, gi)] = bt
                for kc in a_post:
                    _issue_a_chunk(kc)
                return bt

            for _r in range(rep):
                for ci, (e, ngk, fwd, groups) in enumerate(plan):
                    nk = DEPTH[e]
                    act = [j for j in range(NSLOT) if 4 * j < nk]
                    # slot pairs = chunk groups: each pair shares one
                    # [P, 2, CW] psum tile (= one 2KB bank).  start=True
                    # zeroes the whole 2KB zero-region and stop clears its
                    # started flag, so exactly ONE start (first matmul into
                    # the bank, zeroing both lanes) and ONE stop (last
                    # matmul into the bank) per pair.
                    jp = {}
                    pstile = {}
                    pcnt = {}   # pair -> total matmuls
                    pdone = {}  # pair -> matmuls emitted
                    for grp, _ in groups:
                        pair = tuple(sorted(grp))
                        for j in pair:
                            jp[j] = (pair, j - pair[0])
                        pcnt[pair] = sum(
                            1 for kg in range(ngk) for u in range(4)
                            for j in pair if 4 * kg + u >= 4 * j)
                        pdone[pair] = 0
                        if do_mm:
                            pstile[pair] = pspool.tile(
                                [P, 2, CW], mybir.dt.float32, tag="ps",
                                name=f"ps_{_r}_{e}_{pair[0]}")
                    for gi, (grp, _) in enumerate(groups):
                        bt = _load_group(ci, gi)
                        if gi == min(1, len(groups) - 1) and _r == 0 \
                                and ci + 1 < len(plan):
                            # prefetch next col's top group + its A chunk
                            # mid-column, hiding the col-transition load
                            _load_group(ci + 1, 0)
                        w0 = 4 * min(grp)
                        for kg in grp:
                            if do_mm:
                                for u in range(4):
                                    w = 4 * kg + u
                                    c0 = 128 if (kg == ngk - 1 and u == 3) \
                                        else 0
                                    for j in act:
                                        if w < 4 * j:
                                            continue
                                        pair, lane = jp[j]
                                        nc.tensor.matmul(
                                            pstile[pair][:, lane, c0:],
                                            a_sb[:, PRE[w] + j, :],
                                            bt[:, w - w0, c0:],
                                            start=pdone[pair] == 0,
                                            stop=pdone[pair]
                                            == pcnt[pair] - 1)
                                        pdone[pair] += 1
                        if not fwd:
                            _emit_pair(e, tuple(sorted(grp)),
                                       pstile.get(tuple(sorted(grp))),
                                       do_mm, defer=ci < DEFER_COLS)
                    if fwd:
                        for grp, _ in groups:
                            pair = tuple(sorted(grp))
                            _emit_pair(e, pair, pstile.get(pair), do_mm,
                                       defer=ci < DEFER_COLS)
                    if ci == len(plan) - 2 and deferred:
                        for t0_, nl_, ot_ in deferred:
                            _store_pair(t0_, nl_, ot_)
                        deferred.clear()
                    bts.pop((ci, 0), None)
    nc.compile()
    _nc_cache[key] = nc
    return nc


def _inst(c):
    """core -> per-env (q', kshift)"""
    r, s = c % 4, c // 4
    out = []
    for e in range(NENV):
        qp = (15 - 2 * e) if s == 0 else (14 - 2 * e)
        out.append((qp, 0 if s == 0 else 2))
    return r, out


def pack_inputs(A, B, mode=None):
    import ml_dtypes
    A = np.asarray(A, dtype=np.float32)
    B = np.asarray(B, dtype=np.float32)
    bf = ml_dtypes.bfloat16

    in_maps = []
    for c in range(NCORES):
        r, inst = _inst(c)
        ks = inst[0][1]  # kshift (same for all env cols of this core)
        # A pack: position PRE[w]+j holds A[block 4j+r, k=w-ks].T
        ap = np.zeros((ATOT, P, P), np.float32)
        for w in range(32):
            k = w - ks
            if not 0 <= k < 32:
                continue
            for j in range(w // 4 + 1):
                b = 4 * j + r
                if k >= b:
                    ap[PRE[w] + j] = \
                        A[P * b:P * b + P, P * k:P * k + P].T
        apk = np.ascontiguousarray(
            ap.transpose(1, 0, 2)).reshape(P, ATOT * P).astype(bf)

        bp = np.zeros((NENV, P, 32, CW), np.float32)
        for e, (qp, _) in enumerate(inst):
            for w in range(DEPTH[e]):
                k = w - ks
                if not 0 <= k < 32:
                    continue
                if P * k >= CW * (qp + 1):
                    continue  # below diagonal: zero
                bp[e, :, w, :] = \
                    B[P * k:P * k + P, CW * qp:CW * (qp + 1)]
        bpk = np.ascontiguousarray(
            bp.reshape(NENV * P, 32 * CW)).astype(bf)
        in_maps.append({"Apack": apk, "B": bpk})
    return in_maps


def unpack_output(results):
    C = np.zeros((N, N), np.float32)
    for c, r_ in enumerate(results):
        r, inst = _inst(c)
        co = np.asarray(r_["Cout"]).astype(np.float32).reshape(NT, P, CW)
        for t, (e, j) in enumerate(PAIRS):
            b = 4 * j + r
            qp = inst[e][0]
            if P * b >= CW * (qp + 1):
                continue  # entirely below diagonal
            C[P * b:P * b + P, CW * qp:CW * qp + CW] = co[t]
    return C


def kernel(A, B):
    nc = build_nc()
    in_maps = pack_inputs(A, B)
    res = bass_utils.run_bass_kernel_spmd(
        nc, in_maps, core_ids=list(range(NCORES)), trace=False)
    return unpack_output(res.results)


# revision 15
# speedup vs baseline: 1.0057x; 1.0057x over previous
"""v3: 4 row-groups x 2 column-halves at 256-col granularity.

C = triu(A @ B), 4096^2, bf16.  Cores (r, s): r = c % 4, s = c // 4.
16 half-columns q' (256 cols, depth 2q'+2 k-tiles).  s=0 takes odd q',
s=1 even.  Env col e = 0..7 pairs ranks: s=0 -> q' = 15-2e (depth
32-4e = DEPTH[e]), s=1 -> q' = 14-2e (depth 30-4e, front-padded 2 zero
k-steps so its diagonal taper aligns with the schedule's).
Slots j = 0..7, block b = 4j + r, live iff 4j < DEPTH[e].
"""

import numpy as np
from contextlib import ExitStack

import concourse.mybir as mybir
import concourse.tile as tile
from concourse import bacc, bass_utils

N = 4096
P = 128
NCORES = 8
CW = 256
NENV = 8
DEPTH = [32 - 4 * e for e in range(NENV)]
NSLOT = 8
PAIRS = [(e, j) for e in range(NENV) for j in range(NSLOT)
         if 4 * j < DEPTH[e]]
NT = len(PAIRS)                          # 36 output tiles per core
# A-pack, w-major: position(w, j) = PRE[w] + j for j <= w//4
PRE = np.cumsum([0] + [w // 4 + 1 for w in range(32)]).tolist()
ATOT = PRE[32]                           # 144 tiles
ACH = [(PRE[4 * kc], PRE[4 * kc + 4]) for kc in range(8)]

BUFS_B = 10
BUFS_O = 6
BUFS_PS = 8

# env col emission order; first FWD_COLS run forward-k.  Ascending size
# start (small DMA deficit while PE ramps), deep cols last (PE-rich per
# byte, so the tail is compute-bound, not DMA-bound).
I_ORDER = [6, 5, 4, 3, 2, 7, 1, 0]
FWD_COLS = 0
# stores of the first DEFER_COLS cols are held in SBUF and issued at the
# last col's start, keeping the DMA-deficit phase free of store traffic
DEFER_COLS = 0

MODE = "bf16"
_nc_cache = {}


def build_nc(mode=None, rep=1, variant="full"):
    if mode in (1, 2, 4, 8):
        rep, mode = mode, None
    key = (rep, variant, tuple(I_ORDER), FWD_COLS, DEFER_COLS)
    if key in _nc_cache:
        return _nc_cache[key]
    dt_in = mybir.dt.bfloat16
    dt_out = mybir.dt.bfloat16

    nc = bacc.Bacc("TRN2", target_bir_lowering=False, debug=False,
                   num_devices=NCORES)
    a_dram = nc.dram_tensor("Apack", [P, ATOT * P], dt_in,
                            kind="ExternalInput").ap()
    # B-pack row = e*P + p, col = w*CW + n  (w local to env col)
    b_dram = nc.dram_tensor("B", [NENV * P, 32 * CW], dt_in,
                            kind="ExternalInput").ap()
    c_dram = nc.dram_tensor("Cout", [NT * P, CW], dt_out,
                            kind="ExternalOutput").ap()

    with tile.TileContext(nc) as tc:
        with ExitStack() as ctx:
            apool = ctx.enter_context(tc.tile_pool(name="apool", bufs=1))
            bpool = ctx.enter_context(tc.tile_pool(name="bpool", bufs=BUFS_B))
            opool = ctx.enter_context(tc.tile_pool(name="opool", bufs=BUFS_O))
            pspool = ctx.enter_context(
                tc.tile_pool(name="pspool", bufs=BUFS_PS, space="PSUM"))

            do_bdma = variant in ("full", "nomm")
            do_mm = variant in ("full", "nodma")

            a_sb = apool.tile([P, ATOT, P], dt_in)

            # per-col plan: groups of up to 2 chunks in processing order
            plan = []
            seen_a = set()
            for ii, e in enumerate(I_ORDER):
                ngk = DEPTH[e] // 4
                fwd = ii < FWD_COLS
                kgs = list(range(ngk)) if fwd \
                    else list(range(ngk - 1, -1, -1))
                groups = []
                for x in range(0, len(kgs), 2):
                    grp = kgs[x:x + 2]
                    anew = [kg for kg in grp if kg not in seen_a]
                    seen_a.update(anew)
                    groups.append((grp, sorted(anew)))
                plan.append((e, ngk, fwd, groups))

            def _issue_a_chunk(kc):
                t0, t1 = ACH[kc]
                nc.sync.dma_start(
                    a_sb[:, t0:t1, :],
                    a_dram[:, t0 * P:t1 * P].rearrange(
                        "p (t m) -> p t m", m=P))

            def _bw(e, w0, w1):
                # B window [w0, w1) of env col e as one DMA source
                return b_dram[
                    e * P:(e + 1) * P,
                    w0 * CW:w1 * CW,
                ].rearrange("p (ko n) -> p ko n", ko=w1 - w0)

            deferred = []   # (t0, nlane, ot) stores held back

            def _store_pair(t0, nlane, ot):
                if nlane == 2:
                    nc.gpsimd.dma_start(
                        c_dram[t0 * P:(t0 + 2) * P, :].rearrange(
                            "(g p) n -> p g n", g=2), ot[:])
                else:
                    nc.gpsimd.dma_start(
                        c_dram[t0 * P:(t0 + 1) * P, :], ot[:, 0])

            def _emit_pair(e, jpair, pst, do_mm, lane0=0, defer=False):
                """One copy + one store for slots jpair sharing a psum
                bank.  jpair ascending; lanes t-ascending by layout.
                lane0: psum lane of jpair[0]."""
                nlane = len(jpair)
                t0 = PAIRS.index((e, jpair[0]))
                ot = opool.tile([P, nlane, CW], dt_out, tag=f"ot{nlane}")
                if do_mm:
                    nc.vector.tensor_copy(
                        ot[:], pst[:, lane0:lane0 + nlane])
                else:
                    nc.vector.tensor_copy(
                        ot[:].rearrange("p g (a b) -> p g a b", a=2),
                        a_sb[:, :2 * nlane, :].rearrange(
                            "p (g a) m -> p g a m", g=nlane))
                if defer:
                    deferred.append((t0, nlane, ot))
                else:
                    _store_pair(t0, nlane, ot)

            bts = {}       # (ci, gi) -> loaded B tile
            bt_fixed = [None]

            def _load_group(ci, gi, first=False):
                if (ci, gi) in bts:
                    return bts[(ci, gi)]
                e, ngk, fwd, groups = plan[ci]
                grp, anew = groups[gi]
                # A chunk feeding the group's first-processed chunk goes
                # before the B load; the rest after (off the critical path)
                a_pre = [kc for kc in anew if kc == grp[0]]
                a_post = [kc for kc in anew if kc != grp[0]]
                if ci == 0 and gi == 0:
                    # startup: dispatch the critical A chunk on the idle
                    # scalar ring so it overlaps the B load's SP dispatch
                    for kc in a_pre:
                        t0, t1 = ACH[kc]
                        nc.scalar.dma_start(
                            a_sb[:, t0:t1, :],
                            a_dram[:, t0 * P:t1 * P].rearrange(
                                "p (t m) -> p t m", m=P))
                    a_pre = []
                for kc in a_pre:
                    _issue_a_chunk(kc)
                w0 = 4 * min(grp)
                w1 = 4 * max(grp) + 4
                if not do_bdma:
                    if bt_fixed[0] is None:
                        bt_fixed[0] = bpool.tile([P, 8, CW], dt_in,
                                                 tag="bt8")
                        nc.sync.dma_start(bt_fixed[0][:], _bw(e, 0, 8))
                    bts[(ci, gi)] = bt_fixed[0]
                    for kc in a_post:
                        _issue_a_chunk(kc)
                    return bt_fixed[0]
                bt = bpool.tile([P, w1 - w0, CW], dt_in,
                                tag=f"bt{w1 - w0}")
                isdiag = max(grp) == ngk - 1
                if first:
                    # first two k-tiles individually (first matmul waits on
                    # 64KB), rest as one load
                    for u in range(2):
                        nc.sync.dma_start(
                            bt[:, u, :],
                            b_dram[e * P:(e + 1) * P,
                                   (w0 + u) * CW:(w0 + u + 1) * CW])
                    last = w1 - 1 - w0
                    if isdiag:
                        nc.sync.dma_start(bt[:, 2:last, :],
                                          _bw(e, w0 + 2, w1 - 1))
                        nc.sync.dma_start(
                            bt[:, last, 128:],
                            b_dram[e * P:(e + 1) * P,
                                   (w1 - 1) * CW + 128:w1 * CW])
                    else:
                        nc.sync.dma_start(bt[:, 2:, :], _bw(e, w0 + 2, w1))
                elif isdiag:
                    # top k-tile's low 128 cols are below-diag zeros
                    nc.sync.dma_start(bt[:, :w1 - 1 - w0, :],
                                      _bw(e, w0, w1 - 1))
                    nc.sync.dma_start(
                        bt[:, w1 - 1 - w0, 128:],
                        b_dram[e * P:(e + 1) * P,
                               (w1 - 1) * CW + 128:w1 * CW])
                else:
                    nc.sync.dma_start(bt[:], _bw(e, w0, w1))
                bts[(ci, gi)] = bt
                for kc in a_post:
                    _issue_a_chunk(kc)
                return bt

            for _r in range(rep):
                for ci, (e, ngk, fwd, groups) in enumerate(plan):
                    nk = DEPTH[e]
                    act = [j for j in range(NSLOT) if 4 * j < nk]
                    # slot pairs = chunk groups: each pair shares one
                    # [P, 2, CW] psum tile (= one 2KB bank).  start=True
                    # zeroes the whole 2KB zero-region and stop clears its
                    # started flag, so exactly ONE start (first matmul into
                    # the bank, zeroing both lanes) and ONE stop (last
                    # matmul into the bank) per pair.
                    jp = {}
                    pstile = {}
                    pcnt = {}   # pair -> total matmuls
                    pdone = {}  # pair -> matmuls emitted
                    for grp, _ in groups:
                        pair = tuple(sorted(grp))
                        for j in pair:
                            jp[j] = (pair, j - pair[0])
                        pcnt[pair] = sum(
                            1 for kg in range(ngk) for u in range(4)
                            for j in pair if 4 * kg + u >= 4 * j)
                        pdone[pair] = 0
                        if do_mm:
                            pstile[pair] = pspool.tile(
                                [P, 2, CW], mybir.dt.float32, tag="ps",
                                name=f"ps_{_r}_{e}_{pair[0]}")
                    for gi, (grp, _) in enumerate(groups):
                        bt = _load_group(ci, gi)
                        if gi == min(1, len(groups) - 1) and _r == 0 \
                                and ci + 1 < len(plan):
                            # prefetch next col's top group + its A chunk
                            # mid-column, hiding the col-transition load
                            _load_group(ci + 1, 0)
                        w0 = 4 * min(grp)
                        for kg in grp:
                            if do_mm:
                                for u in range(4):
                                    w = 4 * kg + u
                                    c0 = 128 if (kg == ngk - 1 and u == 3) \
                                        else 0
                                    for j in act:
                                        if w < 4 * j:
                                            continue
                                        pair, lane = jp[j]
                                        nc.tensor.matmul(
                                            pstile[pair][:, lane, c0:],
                                            a_sb[:, PRE[w] + j, :],
                                            bt[:, w - w0, c0:],
                                            start=pdone[pair] == 0,
                                            stop=pdone[pair]
                                            == pcnt[pair] - 1)
                                        pdone[pair] += 1
                        if not fwd:
                            _emit_pair(e, tuple(sorted(grp)),
                                       pstile.get(tuple(sorted(grp))),
                                       do_mm, defer=ci < DEFER_COLS)
                    if fwd:
                        for grp, _ in groups:
                            pair = tuple(sorted(grp))
                            _emit_pair(e, pair, pstile.get(pair), do_mm,
                                       defer=ci < DEFER_COLS)
                    if ci == len(plan) - 2 and deferred:
                        for t0_, nl_, ot_ in deferred:
                            _store_pair(t0_, nl_, ot_)
                        deferred.clear()
                    bts.pop((ci, 0), None)
    nc.compile()
    _nc_cache[key] = nc
    return nc


def _inst(c):
    """core -> per-env (q', kshift)"""
    r, s = c % 4, c // 4
    out = []
    for e in range(NENV):
        qp = (15 - 2 * e) if s == 0 else (14 - 2 * e)
        out.append((qp, 0 if s == 0 else 2))
    return r, out


def pack_inputs(A, B, mode=None):
    import ml_dtypes
    A = np.asarray(A, dtype=np.float32)
    B = np.asarray(B, dtype=np.float32)
    bf = ml_dtypes.bfloat16

    in_maps = []
    for c in range(NCORES):
        r, inst = _inst(c)
        ks = inst[0][1]  # kshift (same for all env cols of this core)
        # A pack: position PRE[w]+j holds A[block 4j+r, k=w-ks].T
        ap = np.zeros((ATOT, P, P), np.float32)
        for w in range(32):
            k = w - ks
            if not 0 <= k < 32:
                continue
            for j in range(w // 4 + 1):
                b = 4 * j + r
                if k >= b:
                    ap[PRE[w] + j] = \
                        A[P * b:P * b + P, P * k:P * k + P].T
        apk = np.ascontiguousarray(
            ap.transpose(1, 0, 2)).reshape(P, ATOT * P).astype(bf)

        bp = np.zeros((NENV, P, 32, CW), np.float32)
        for e, (qp, _) in enumerate(inst):
            for w in range(DEPTH[e]):
                k = w - ks
                if not 0 <= k < 32:
                    continue
                if P * k >= CW * (qp + 1):
                    continue  # below diagonal: zero
                bp[e, :, w, :] = \
                    B[P * k:P * k + P, CW * qp:CW * (qp + 1)]
        bpk = np.ascontiguousarray(
            bp.reshape(NENV * P, 32 * CW)).astype(bf)
        in_maps.append({"Apack": apk, "B": bpk})
    return in_maps


def unpack_output(results):
    C = np.zeros((N, N), np.float32)
    for c, r_ in enumerate(results):
        r, inst = _inst(c)
        co = np.asarray(r_["Cout"]).astype(np.float32).reshape(NT, P, CW)
        for t, (e, j) in enumerate(PAIRS):
            b = 4 * j + r
            qp = inst[e][0]
            if P * b >= CW * (qp + 1):
                continue  # entirely below diagonal
            C[P * b:P * b + P, CW * qp:CW * qp + CW] = co[t]
    return C


def kernel(A, B):
    nc = build_nc()
    in_maps = pack_inputs(A, B)
    res = bass_utils.run_bass_kernel_spmd(
        nc, in_maps, core_ids=list(range(NCORES)), trace=False)
    return unpack_output(res.results)
